# revision 17
# baseline (speedup 1.0000x reference)
"""Bahdanau-attention scoring kernel for 8 TRN2 NeuronCores.

Reference computation (S=2048, B=32, H=1024):
    cat    = concat([broadcast(hidden), enc], axis=2)          # [S,B,2H]
    alphas = tanh(einsum('sbk,hk->sbh', cat, W_attn) + b_attn) # [S,B,H]
    scores = einsum('sbh,h->sb', alphas, v)                    # [S,B]
    out    = softmax(scores.T, axis=1)[:, None, :]             # [B,1,S]

Because hidden is broadcast over S, the concat-matmul splits into
    z[s,b,:] = W2 @ enc[s,b,:] + (W1 @ hidden[b,:] + b_attn)
with W1 = W_attn[:, :H], W2 = W_attn[:, H:].  The per-batch term hp[b,:]
is a [H]-vector, computed once on device and broadcast to all partitions.

Layout: s on partitions, h on the free dim.  Per 128-row s-tile:
  z = et.T @ W2T-slice (8 accumulating matmuls, enc tile stationary),
  zq = z + hp[b] (DVE), alphas = tanh(zq) (ACT),
  scores partial = sum_h alphas*v (DVE tensor_tensor_reduce).
This keeps the TensorEngine to exactly the 137 GFLOP main matmul; the
v-contraction and bias ride on the otherwise-idle Vector engine.

Sharding: data-parallel over batch. Core c handles batches 4c..4c+3.
"""

import sys

for _p in ("/opt/trn_rl_repo", "/root/.axon_site/_ro/trn_rl_repo"):
    if _p not in sys.path:
        sys.path.insert(0, _p)

import numpy as np
import ml_dtypes

import concourse.bass as bass  # noqa: F401  (bass must import before tile)
import concourse.mybir as mybir
import concourse.tile as tile
from concourse import bacc
from concourse.bass_isa import ReduceOp
from concourse.bass_utils import run_bass_kernel_spmd

S, B, H = 2048, 32, 1024
NCORES = 8
BL = B // NCORES          # batches per core (4)
P = 128                   # SBUF partitions
HT = H // P               # h k-tiles (8)
SC = 512                  # s-chunk per enc DMA
NSC = S // SC             # s chunks per batch row (4)
ST = SC // P              # s-tiles per chunk (4)
HC = H // 512             # h output chunks (2)

BF16 = mybir.dt.bfloat16
F32 = mybir.dt.float32
AFT = mybir.ActivationFunctionType
MUL = mybir.AluOpType.mult
ADD = mybir.AluOpType.add

_nc_cache = None


def build():
    nc = bacc.Bacc()
    enc = nc.declare_dram_parameter("enc", [BL, H, S], BF16, isOutput=False)
    wt = nc.declare_dram_parameter("wt", [2 * H, H], BF16, isOutput=False)
    hid = nc.declare_dram_parameter("hid", [H, BL], BF16, isOutput=False)
    ba = nc.declare_dram_parameter("ba", [1, H], BF16, isOutput=False)
    vv = nc.declare_dram_parameter("v", [1, H], BF16, isOutput=False)
    # out[p, b*16 + sc*4 + st] = softmax row b at s = sc*512 + st*128 + p
    out = nc.declare_dram_parameter("out", [P, BL * NSC * ST], F32, isOutput=True)

    with tile.TileContext(nc) as tc:
        with (
            tc.tile_pool(name="const", bufs=1) as cpool,
            tc.tile_pool(name="encp", bufs=4) as encp,
            tc.tile_pool(name="zqp", bufs=8) as zqp,
            tc.tile_pool(name="alqp", bufs=8) as alqp,
            tc.tile_pool(name="prodp", bufs=4) as prodp,
            tc.tile_pool(name="smallp", bufs=8) as smallp,
            tc.tile_pool(name="zps", bufs=7, space="PSUM") as zps,
            tc.tile_pool(name="hpps", bufs=1, space="PSUM") as hpps,
        ):
            # --- resident constants ---
            hid_sb = cpool.tile([P, HT, BL], BF16)
            nc.gpsimd.dma_start(hid_sb[:], hid.rearrange("(t p) b -> p t b", p=P))
            ba_sb = cpool.tile([1, H], BF16)
            nc.gpsimd.dma_start(ba_sb[:], ba[:])
            v_row = cpool.tile([1, H], BF16)
            nc.gpsimd.dma_start(v_row[:], vv[:])
            ones1 = cpool.tile([1, BL], BF16)
            nc.vector.memset(ones1[:], 1.0)
            # sync-queue order: enc chunk0, W2 (z needs it first), W1 (only
            # the hp projection needs it) -- so the z matmuls start earliest
            et0 = encp.tile([P, HT, SC], BF16, tag="enc")
            w_sb = cpool.tile([P, 2 * HT, H], BF16)   # W1T | W2T, k on partitions
            for kt in range(HT):   # pairwise: z matmul kt needs both pieces
                nc.sync.dma_start(w_sb[:, HT + kt, :],
                                  wt[(HT + kt) * P:(HT + kt + 1) * P, :])
                nc.sync.dma_start(et0[:, kt, :], enc[0, kt * P:(kt + 1) * P, 0:SC])
            # W1 streams on the gpsimd queue in parallel (hp needs all of it
            # before the first tanh; sync is busy with W2+enc0)
            for t in range(HT):
                nc.gpsimd.dma_start(w_sb[:, t, :], wt[t * P:(t + 1) * P, :])
            v_bc = cpool.tile([P, H], BF16)
            nc.gpsimd.partition_broadcast(v_bc[:], v_row[:])

            # --- per-batch bias row: hp[b, :] = W1 @ hidden[b] + b_attn ---
            # (emitted after chunk 0's matmuls: W1 arrives after W2)
            hp_bc = []

            def emit_hp():
                hpb_sb = cpool.tile([BL, H], F32)
                for hc in range(HC):
                    hp_ps = hpps.tile([BL, 512], F32, tag="hp")
                    for kt in range(HT):
                        nc.tensor.matmul(
                            hp_ps[:], hid_sb[:, kt, :],
                            w_sb[:, kt, hc * 512:(hc + 1) * 512],
                            start=(kt == 0), stop=False,
                        )
                    # + b_attn as a K=1 rank-1 update (ones ⊗ ba)
                    nc.tensor.matmul(
                        hp_ps[:], ones1[:], ba_sb[:, hc * 512:(hc + 1) * 512],
                        start=False, stop=True,
                    )
                    nc.scalar.copy(hpb_sb[:, hc * 512:(hc + 1) * 512], hp_ps[:])
                hpb16 = cpool.tile([BL, H], BF16)
                nc.vector.tensor_copy(hpb16[:], hpb_sb[:])
                for b in range(BL):
                    # engines can only address partition bases {0,32,64,96};
                    # DMA the row to partition 0 first, then broadcast
                    row = cpool.tile([1, H], BF16, tag=f"hprow{b}")
                    nc.sync.dma_start(row[:], hpb16[b:b + 1, :])
                    t = cpool.tile([P, H], BF16, tag=f"hpbc{b}")
                    nc.gpsimd.partition_broadcast(t[:], row[:])
                    hp_bc.append(t)

            # --- main loop ---
            scores_sb = cpool.tile([P, BL * NSC * ST], F32)
            ex_sb = cpool.tile([P, BL * NSC * ST], F32)
            osb = cpool.tile([P, BL * NSC * ST], F32)
            first = True
            for b in range(BL):
                for sc in range(NSC):
                    if first:
                        et = et0
                        emit_hp()
                        first = False
                    else:
                        et = encp.tile([P, HT, SC], BF16, tag="enc")
                        for kt in range(HT):
                            nc.sync.dma_start(
                                et[:, kt, :],
                                enc[b, kt * P:(kt + 1) * P,
                                    sc * SC:(sc + 1) * SC],
                            )
                    # chunk 0 arrives while W2/enc stream in: emit its
                    # matmuls kt-major in waves of 4 units so each arriving
                    # (W2[kt], enc[kt]) pair feeds 4 back-to-back matmuls
                    if b == 0 and sc == 0:
                        zw = {}
                        for hc in range(HC):
                            for st in range(ST):
                                zw[(st, hc)] = zps.tile(
                                    [P, 512], F32, tag="z", name=f"zw{st}_{hc}")
                            for kt in range(HT):
                                for st in range(ST):
                                    nc.tensor.matmul(
                                        zw[(st, hc)][:],
                                        et[:, kt, st * P:(st + 1) * P],
                                        w_sb[:, HT + kt,
                                             hc * 512:(hc + 1) * 512],
                                        start=(kt == 0), stop=(kt == HT - 1),
                                    )
                    for st in range(ST):
                        parts = []
                        for hc in range(HC):
                            if b == 0 and sc == 0:
                                z_ps = zw[(st, hc)]
                            else:
                                z_ps = zps.tile([P, 512], F32, tag="z")
                                for kt in range(HT):
                                    nc.tensor.matmul(
                                        z_ps[:],
                                        et[:, kt, st * P:(st + 1) * P],
                                        w_sb[:, HT + kt,
                                             hc * 512:(hc + 1) * 512],
                                        start=(kt == 0), stop=(kt == HT - 1),
                                    )
                            zq = zqp.tile([P, 512], BF16, tag="zq")
                            nc.vector.tensor_add(
                                zq[:], z_ps[:], hp_bc[b][:, hc * 512:(hc + 1) * 512])
                            alq = alqp.tile([P, 512], BF16, tag="alq")
                            nc.scalar.activation(alq[:], zq[:], AFT.Tanh)
                            # fused multiply+reduce on DVE; the elementwise
                            # result is discarded via a step-0 dummy out
                            # (tensor_tensor_reduce crashes this runtime)
                            dummy = prodp.tile([P, 1], BF16, tag="prod")
                            part = smallp.tile([P, 1], F32, tag="part")
                            nc.vector.scalar_tensor_tensor(
                                dummy.broadcast_to(alq.shape), alq[:], 1.0,
                                v_bc[:, hc * 512:(hc + 1) * 512],
                                op0=MUL, op1=MUL, accum_out=part[:])
                            parts.append(part)
                        col = (b * NSC + sc) * ST + st
                        nc.vector.tensor_add(
                            scores_sb[:, col:col + 1], parts[0][:], parts[1][:])

                # --- softmax row b (no max-sub: |scores| <= sum|v| ~ 26) ---
                cs = slice(b * NSC * ST, (b + 1) * NSC * ST)
                psum_row = smallp.tile([P, 1], F32, tag="psrow")
                nc.scalar.activation(
                    ex_sb[:, cs], scores_sb[:, cs], AFT.Exp, accum_out=psum_row[:])
                tot = smallp.tile([P, 1], F32, tag="tot")
                nc.gpsimd.partition_all_reduce(
                    tot[:], psum_row[:], P, ReduceOp.add)
                rec = smallp.tile([P, 1], F32, tag="rec")
                nc.vector.reciprocal(rec[:], tot[:])
                nc.vector.tensor_scalar_mul(osb[:, cs], ex_sb[:, cs], rec[:, 0:1])
                nc.sync.dma_start(out[:, cs], osb[:, cs])
    nc.compile()
    return nc


def _get_nc():
    global _nc_cache
    if _nc_cache is None:
        _nc_cache = build()
    return _nc_cache


def kernel(hidden, encoder_outputs, W_attn, b_attn, v, _trace=False):
    bf16 = ml_dtypes.bfloat16
    hidden = np.asarray(hidden, dtype=np.float32)
    encoder_outputs = np.asarray(encoder_outputs, dtype=np.float32)
    W_attn = np.asarray(W_attn, dtype=np.float32)
    b_attn = np.asarray(b_attn, dtype=np.float32)
    v = np.asarray(v, dtype=np.float32)

    wt = W_attn.T.astype(bf16)                     # [2H, H] contiguous
    hid_t = hidden[0].T.astype(bf16)               # [H, B]
    ba = b_attn.reshape(1, H).astype(bf16)
    vv = v.reshape(1, H).astype(bf16)
    # [B, H, S] b-major, s-contiguous
    enc_t = encoder_outputs.transpose(1, 2, 0).astype(bf16)

    in_maps = []
    for c in range(NCORES):
        bsl = slice(c * BL, (c + 1) * BL)
        in_maps.append({
            "enc": np.ascontiguousarray(enc_t[bsl]),
            "wt": wt,
            "hid": np.ascontiguousarray(hid_t[:, bsl]),
            "ba": ba,
            "v": vv,
        })

    nc = _get_nc()
    res = run_bass_kernel_spmd(
        nc, in_maps, core_ids=list(range(NCORES)), trace=_trace,
    )
    # out[p, b*16 + sc*4 + st] -> [b, s = sc*512 + st*128 + p]
    parts = []
    for c in range(NCORES):
        r = res.results[c]["out"].reshape(P, BL, NSC, ST)
        parts.append(r.transpose(1, 2, 3, 0).reshape(BL, S))
    full = np.concatenate(parts, axis=0)
    out = full[:, None, :].astype(np.float32)      # [B, 1, S]
    if _trace:
        return out, res
    return out
